# revision 3
# baseline (speedup 1.0000x reference)
"""Trainium2 Bass kernel for nn_MeshDownConv (2-layer SplineConv GNN).

Sharding: 8 cores = 4 meshes x 2 dst-halves. Host relabels nodes so each
128-node "window" has a near-uniform in-edge count, sorts edges by window,
and precomputes the 9 B-spline basis values per edge. Edges in a window are
split into two passes by source half (A: src < NHALF, B: src >= NHALF) so
dma_gather's int16 indices can address the table. Device pipeline per
window: dma_gather of source features (edge-partitioned, 256B rows), one
DVE tensor_tensor per pass builds u[e, (k,c)] = basis_k[e] * xj[e, c] via
step-0 broadcast APs, one DVE tensor_tensor builds the one-hot incidence
inc[e, n] = (dstrel[e] == n), then 18 PE matmuls contract the edge dim:
z[n, (k,c)] += inc^T @ u  (PSUM accumulation). Node-side: transpose z,
multiply by Wflat chunks, add root term + bias, relu. Halves exchange
node features with a 2-core AllGather between the layers.
"""
import sys

sys.path.insert(0, "/opt/trn_rl_repo")

import numpy as np

import concourse.bass as bass
import concourse.mybir as mybir
from concourse import bacc, tile, bass_utils

F32 = mybir.dt.float32
I16 = mybir.dt.int16
I32 = mybir.dt.int32


class CFG:
    C = 32            # in channels
    O = 32            # out channels
    KK = 9            # spline kernels

    @property
    def RL(self):
        # table row length: 256B granule for dma_gather
        return 64 if self.DT == F32 else 128
    NW = 196          # windows per half
    NCHA = 9          # chunks (of 128 edges) per window per pass
    GW = 4            # windows per gather group
    N = 50000         # real nodes per mesh
    E = 800000        # edges per mesh
    B = 4             # meshes
    NCORES = 8
    DT = mybir.dt.float16   # edge-side dtype (table/xj/u/inc/basis)
    UPOOL = 5               # u-build chunks offloaded to gpsimd (of NCH)
    INCPOOL = False         # build incidence on gpsimd (else DVE)

    @property
    def NCH(self):
        return 2 * self.NCHA

    @property
    def NHALF(self):
        return self.NW * 128

    @property
    def NN(self):
        return 2 * self.NHALF

    @property
    def NCHT(self):
        return self.NW * self.NCH

    @property
    def NSLOTA(self):
        # gather slots per half-core per pass
        return self.NW * self.NCHA * 128


def _np_dt(dt):
    return {F32: np.float32, mybir.dt.float16: np.float16}[dt]


# ----------------------------------------------------------------- host prep

def _quad_basis_np(t):
    return np.stack([0.5 * (1.0 - t) ** 2, -t * t + t + 0.5, 0.5 * t * t],
                    axis=-1)


def _balance_nodes(deg, nbins, cap_nodes=128):
    """Greedy: assign nodes (desc by degree) to the lightest non-full bin.
    Returns sigma: old -> new id (bin*128 + slot)."""
    import heapq
    n = deg.shape[0]
    order = np.argsort(-deg, kind="stable")
    heap = [(0, b) for b in range(nbins)]
    heapq.heapify(heap)
    counts = np.zeros(nbins, np.int64)
    sums = np.zeros(nbins, np.int64)
    sigma = np.empty(n, np.int64)
    for old in order:
        while True:
            s, b = heapq.heappop(heap)
            if counts[b] < cap_nodes:
                break
        sigma[old] = b * 128 + counts[b]
        counts[b] += 1
        sums[b] += deg[old]
        if counts[b] < cap_nodes:
            heapq.heappush(heap, (sums[b], b))
    return sigma, sums


def _host_prep_mesh(cfg, x, edge, pseudo):
    """Per-mesh host preprocessing.

    Returns (tab0 [NN, RL] f32, halves, sigma) where halves[h] =
    (IDXA [16, NSLOTA/16] i16, IDXB, ED [128, NCHT, 10] DT).
    """
    npdt = _np_dt(cfg.DT)
    src, dst = edge[0].astype(np.int64), edge[1].astype(np.int64)
    E = src.shape[0]

    B0 = _quad_basis_np(pseudo[:, 0].astype(np.float32))
    B1 = _quad_basis_np(pseudo[:, 1].astype(np.float32))
    basis = (B1[:, :, None] * B0[:, None, :]).reshape(E, cfg.KK)

    deg = np.bincount(dst, minlength=cfg.N)
    sigma, sums = _balance_nodes(deg, 2 * cfg.NW)

    new_dst = sigma[dst]
    new_src = sigma[src]

    tab0 = np.zeros((cfg.NN, cfg.RL), _np_dt(cfg.DT))
    tab0[sigma[np.arange(cfg.N)], :cfg.C] = x.astype(_np_dt(cfg.DT))

    # order edges by (window, src-half)
    win = new_dst // 128
    srchalf = (new_src >= cfg.NHALF).astype(np.int64)
    key = win * 2 + srchalf
    order = np.argsort(key, kind="stable")
    key_sorted = key[order]
    starts = np.searchsorted(key_sorted, np.arange(4 * cfg.NW + 1))
    capa = cfg.NCHA * 128
    halves = []
    for h in range(2):
        IDX = np.zeros((2, 16, cfg.NSLOTA // 16), np.int16)  # wrapped
        ED = np.zeros((128, cfg.NCHT, 1 + cfg.KK), np.float32)
        for wl in range(cfg.NW):
            w = h * cfg.NW + wl
            for p in range(2):  # pass A/B
                lo, hi = starts[2 * w + p], starts[2 * w + p + 1]
                eids = order[lo:hi]
                cnt = hi - lo
                assert cnt <= capa, (cnt, capa, w, p)
                slot = np.arange(cnt)
                ch = (p * cfg.NCHA + wl * cfg.NCH) + slot // 128
                lane = slot % 128
                ED[lane, ch, 0] = (new_dst[eids] - w * 128).astype(np.float32)
                ED[lane, ch, 1:] = basis[eids]
                gslot = (wl * cfg.NCHA + slot // 128) * 128 + lane
                sidx = new_src[eids] - p * cfg.NHALF
                IDX[p, gslot % 16, gslot // 16] = sidx.astype(np.int16)
        halves.append((np.tile(IDX[0], (8, 1)), np.tile(IDX[1], (8, 1)),
                       ED.astype(npdt)))
    return tab0, halves, sigma


# ------------------------------------------------------------- bass program

def _ap(t, offset, pattern):
    base = t if isinstance(t, bass.AP) else t[:]
    return bass.AP(base.tensor, base.offset + offset, pattern)


def _sap(t, offset, freedims, npart=None):
    """Slice of an SBUF/PSUM tile in the flat AP convention."""
    base = t if isinstance(t, bass.AP) else t[:]
    p = base.ap[0]
    part = [p[0], p[1] if npart is None else npart]
    return bass.AP(base.tensor, base.offset + offset, [part] + list(freedims))


def build_program(cfg, ncores, sim_single=False):
    nc = bacc.Bacc("TRN2", target_bir_lowering=False, debug=False,
                   num_devices=ncores)
    DT = cfg.DT
    C, O, KK, RL = cfg.C, cfg.O, cfg.KK, cfg.RL
    NCH, NCHA, GW, NW = cfg.NCH, cfg.NCHA, cfg.GW, cfg.NW
    NG = NW // GW
    UD = KK * C   # 288
    NI16 = cfg.NSLOTA // 16

    tab0 = nc.dram_tensor("tab0", [cfg.NN, RL], DT, kind="ExternalInput")
    xown0 = nc.dram_tensor("xown0", [cfg.NHALF, RL], DT,
                           kind="ExternalInput")
    idxa = nc.dram_tensor("idxa", [128, NI16], I16, kind="ExternalInput")
    idxb = nc.dram_tensor("idxb", [128, NI16], I16, kind="ExternalInput")
    edd = nc.dram_tensor("edd", [128, cfg.NCHT, 1 + KK], DT,
                         kind="ExternalInput")
    wf1 = nc.dram_tensor("wf1", [96, 96], F32, kind="ExternalInput")
    wf2 = nc.dram_tensor("wf2", [96, 96], F32, kind="ExternalInput")
    rt1 = nc.dram_tensor("rt1", [C, O], DT, kind="ExternalInput")
    rt2 = nc.dram_tensor("rt2", [C, O], DT, kind="ExternalInput")
    bb1 = nc.dram_tensor("bb1", [O, 1], F32, kind="ExternalInput")
    bb2 = nc.dram_tensor("bb2", [O, 1], F32, kind="ExternalInput")
    outt = nc.dram_tensor("out", [cfg.NHALF, C], F32, kind="ExternalOutput")

    replica_groups = [[2 * i, 2 * i + 1] for i in range(ncores // 2)]

    with tile.TileContext(nc, num_cores=ncores) as tc:
        with tc.tile_pool(name="const", bufs=1) as cpool, \
             tc.tile_pool(name="dram", bufs=1, space="DRAM") as dpool, \
             tc.tile_pool(name="work", bufs=2) as wpool, \
             tc.tile_pool(name="psum", bufs=1, space="PSUM") as ppool:

            from concourse import library_config
            nc.gpsimd.load_library(library_config.mlp)

            # ---- constants
            iotai = cpool.tile([128, 128], I32, name="iotai")
            nc.gpsimd.iota(iotai[:], pattern=[[1, 128]], base=0,
                           channel_multiplier=0)
            iotaf = cpool.tile([128, 128], DT, name="iotaf")
            nc.vector.tensor_copy(iotaf[:], iotai[:])
            idximp = cpool.tile([128, 128], I32, name="idximp")
            nc.gpsimd.iota(idximp[:], pattern=[[1, 128]], base=0,
                           channel_multiplier=-1)
            identf = cpool.tile([128, 128], F32, name="identf")
            nc.vector.tensor_scalar(out=identf[:], in0=idximp[:],
                                    scalar1=0, scalar2=None,
                                    op0=mybir.AluOpType.is_equal)
            identd = identf
            if DT != F32:
                identd = cpool.tile([128, 128], DT, name="identd")
                nc.vector.tensor_copy(identd[:], identf[:])
            zrowa = cpool.tile([1, 128], DT, name="zrowa")
            nc.vector.memset(zrowa[:], 0)
            zrowb = cpool.tile([1, UD], DT, name="zrowb")
            nc.vector.memset(zrowb[:], 0)

            # index tables stay resident in SBUF
            sidxa = cpool.tile([128, NI16], I16, name="sidxa")
            nc.sync.dma_start(sidxa[:], idxa.ap())
            sidxb = cpool.tile([128, NI16], I16, name="sidxb")
            nc.sync.dma_start(sidxb[:], idxb.ap())

            wfs = [cpool.tile([96, 96], F32, name=f"wfs{i}") for i in range(2)]
            rts = [cpool.tile([C, O], DT, name=f"rts{i}") for i in range(2)]
            bbs = [cpool.tile([O, 1], F32, name=f"bbs{i}") for i in range(2)]
            for i, (wsrc, rsrc, bsrc) in enumerate(
                    [(wf1, rt1, bb1), (wf2, rt2, bb2)]):
                nc.sync.dma_start(wfs[i][:], wsrc[:])
                nc.sync.dma_start(rts[i][:], rsrc[:])
                nc.sync.dma_start(bbs[i][:], bsrc[:])

            # ---- internal dram
            town = dpool.tile([cfg.NHALF, RL], DT, name="town")
            tfull = dpool.tile([cfg.NN, RL], DT, name="tfull")

            def layer(tabfull, xown, wfsb, rtsb, bbsb, rows_out, last):
                for g in range(NG):
                    nsg = GW * NCHA * 128       # gather idxs per pass
                    xjt = [None, None]
                    for p, sidx in ((0, sidxa), (1, sidxb)):
                        xj = wpool.tile([128, GW * NCHA * RL], DT,
                                        name=f"xj{p}", bufs=2)
                        # custom-DMA SBUF APs use the flat convention:
                        # partition step = row length in elements
                        nc.gpsimd.dma_gather(
                            out_ap=_sap(xj, 0, [[RL, GW * NCHA],
                                                [1, RL]]),
                            in_ap=_ap(tabfull, p * cfg.NHALF * RL,
                                      [[RL, cfg.NHALF], [1, RL]]),
                            idxs_ap=_sap(sidx, g * nsg // 16,
                                         [[1, nsg // 16]]),
                            num_idxs=nsg,
                            num_idxs_reg=nsg,
                            elem_size=RL,
                            single_packet=False,
                        )
                        xjt[p] = xj
                    edt = wpool.tile([128, GW * NCH * (1 + KK)], DT,
                                     name="edt", bufs=2)
                    nc.sync.dma_start(
                        edt[:],
                        _ap(edd.ap(), g * GW * NCH * (1 + KK),
                            [[cfg.NCHT * (1 + KK), 128],
                             [1, GW * NCH * (1 + KK)]]))
                    xwing = wpool.tile([128, GW * C], DT, name="xwing",
                                       bufs=2)
                    nc.sync.dma_start(
                        xwing[:],
                        _ap(xown, g * GW * 128 * RL,
                            [[RL, 128], [128 * RL, GW], [1, C]]))
                    out_dt = F32 if last else DT
                    rowsg = wpool.tile([128, GW * C], out_dt, name="rowsg",
                                       bufs=2)

                    for wl in range(GW):
                        w = g * GW + wl
                        u = wpool.tile([128, NCH * UD], DT, name="u", bufs=2)
                        for p in range(2):
                            # give the tail of pass B's chunks to gpsimd
                            npool = min(cfg.UPOOL, NCHA) if p == 1 else 0
                            ndve = NCHA - npool
                            if ndve:
                                nc.vector.tensor_tensor(
                                    out=_sap(u, p * NCHA * UD,
                                             [[UD, ndve], [C, KK], [1, C]]),
                                    in0=_sap(xjt[p], wl * NCHA * RL,
                                             [[RL, ndve], [0, KK], [1, C]]),
                                    in1=_sap(edt,
                                             (wl * NCH + p * NCHA) * (1 + KK)
                                             + 1,
                                             [[1 + KK, ndve], [1, KK],
                                              [0, C]]),
                                    op=mybir.AluOpType.mult)
                            if npool:
                                nc.gpsimd.tensor_tensor(
                                    out=_sap(u, (p * NCHA + ndve) * UD,
                                             [[UD, npool], [C, KK], [1, C]]),
                                    in0=_sap(xjt[p],
                                             (wl * NCHA + ndve) * RL,
                                             [[RL, npool], [0, KK], [1, C]]),
                                    in1=_sap(edt,
                                             (wl * NCH + p * NCHA + ndve)
                                             * (1 + KK) + 1,
                                             [[1 + KK, npool], [1, KK],
                                              [0, C]]),
                                    op=mybir.AluOpType.mult)
                        inc = wpool.tile([128, NCH * 128], DT, name="inc",
                                         bufs=2)
                        inc_eng = nc.gpsimd if cfg.INCPOOL else nc.vector
                        inc_eng.tensor_tensor(
                            out=_sap(inc, 0, [[128, NCH], [1, 128]]),
                            in0=_sap(iotaf, 0, [[0, NCH], [1, 128]]),
                            in1=_sap(edt, wl * NCH * (1 + KK),
                                     [[1 + KK, NCH], [0, 128]]),
                            op=mybir.AluOpType.is_equal)

                        z = ppool.tile([128, UD], F32, name="z", bufs=2)
                        for c in range(NCH):
                            # every chunk matmul writes all of z densely, so
                            # no zero-init is needed
                            nc.tensor.matmul(
                                z[:],
                                _sap(inc, c * 128, [[1, 128]]),
                                _sap(u, c * UD, [[1, UD]]),
                                start=(c == 0), stop=(c == NCH - 1))

                        zsb = wpool.tile([128, UD], F32, name="zsb", bufs=2)
                        nc.scalar.copy(zsb[:], z[:])
                        zt = ppool.tile([96, 384], F32, name="zt", bufs=2)
                        for j in range(3):
                            nc.tensor.transpose(
                                _sap(zt, j * 128, [[1, 128]]),
                                _sap(zsb, j * 96, [[1, 96]]),
                                identf[:])
                        ztsb = wpool.tile([96, 384], F32, name="ztsb", bufs=2)
                        nc.scalar.copy(ztsb[:], zt[:])

                        agg = ppool.tile([O, 128], F32, name="agg", bufs=1)
                        for j in range(3):
                            nc.tensor.matmul(
                                agg[:],
                                _sap(wfsb, j * 32, [[1, 32]]),
                                _sap(ztsb, j * 128, [[1, 128]]),
                                start=(j == 0), stop=False)
                        xt = ppool.tile([C, 128], DT, name="xt", bufs=1)
                        nc.tensor.transpose(
                            xt[:],
                            _sap(xwing, wl * C, [[1, C]]),
                            identd[:])
                        xtsb = wpool.tile([C, 128], DT, name="xtsb", bufs=2)
                        nc.scalar.copy(xtsb[:], xt[:])
                        nc.tensor.matmul(agg[:], rtsb[:], xtsb[:],
                                         start=False, stop=True)
                        ht = wpool.tile([O, 128], out_dt, name="ht", bufs=2)
                        nc.scalar.activation(
                            ht[:], agg[:],
                            mybir.ActivationFunctionType.Relu,
                            bias=bbsb[:], scale=1.0)
                        rows = ppool.tile([128, O], out_dt, name="rows",
                                          bufs=1)
                        nc.tensor.transpose(
                            rows[:], ht[:],
                            _sap(identf if last else identd, 0,
                                 [[1, 32]], npart=32))
                        nc.scalar.copy(
                            _sap(rowsg, wl * C, [[1, C]]), rows[:])

                    if last:
                        nc.sync.dma_start(
                            _ap(rows_out, g * GW * 128 * C,
                                [[C, 128], [128 * C, GW], [1, C]]),
                            rowsg[:])
                    else:
                        nc.sync.dma_start(
                            _ap(rows_out, g * GW * 128 * RL,
                                [[RL, 128], [128 * RL, GW], [1, C]]),
                            rowsg[:])

            # layer 1
            layer(tab0.ap(), xown0.ap(), wfs[0], rts[0], bbs[0],
                  town[:], last=False)
            if sim_single:
                # collective-free stand-in for single-core timeline sim:
                # same bytes moved (NHALF rows in + out per core)
                nc.sync.dma_start(
                    _ap(tfull[:], 0, [[1, cfg.NHALF * RL]]),
                    _ap(town[:], 0, [[1, cfg.NHALF * RL]]))
                nc.sync.dma_start(
                    _ap(tfull[:], cfg.NHALF * RL, [[1, cfg.NHALF * RL]]),
                    _ap(town[:], 0, [[1, cfg.NHALF * RL]]))
            else:
                nc.gpsimd.collective_compute(
                    "AllGather", mybir.AluOpType.bypass,
                    replica_groups=replica_groups,
                    ins=[town[:]], outs=[tfull[:]])
            # layer 2
            layer(tfull[:], town[:], wfs[1], rts[1], bbs[1],
                  outt.ap(), last=True)

    nc.finalize()
    return nc


# ------------------------------------------------------------------- driver

_cache = {}


def _get_program(cfg):
    key = (cfg.NW, cfg.NCHA, cfg.GW, cfg.DT, cfg.NCORES,
           cfg.UPOOL, cfg.INCPOOL)
    if key not in _cache:
        _cache[key] = build_program(cfg, cfg.NCORES)
    return _cache[key]


def run(cfg, images, edges, pseudo, W1, root1, b1, W2, root2, b2,
        trace=False, trace_out=None, tmpdir=None):
    wf = []
    for W in (W1, W2):
        Wflat = np.asarray(W, np.float32).reshape(cfg.KK * cfg.C, cfg.O)
        wfl = np.zeros((96, 96), np.float32)
        for j in range(3):
            wfl[:, 32 * j:32 * j + 32] = Wflat[96 * j:96 * j + 96, :]
        wf.append(wfl)
    rts = [np.asarray(r, np.float32).astype(_np_dt(cfg.DT))
           for r in (root1, root2)]
    bbs = [np.asarray(b, np.float32).reshape(cfg.O, 1) for b in (b1, b2)]

    in_maps = []
    sigmas = []
    for b in range(cfg.B):
        tab0, halves, sigma = _host_prep_mesh(
            cfg, np.asarray(images[b], np.float32),
            np.asarray(edges[b]), np.asarray(pseudo[b], np.float32))
        sigmas.append(sigma)
        for h in range(2):
            IDXA, IDXB, ED = halves[h]
            in_maps.append({
                "tab0": tab0,
                "xown0": tab0[h * cfg.NHALF:(h + 1) * cfg.NHALF],
                "idxa": IDXA, "idxb": IDXB,
                "edd": ED,
                "wf1": wf[0], "wf2": wf[1],
                "rt1": rts[0], "rt2": rts[1],
                "bb1": bbs[0], "bb2": bbs[1],
            })

    nc = _get_program(cfg)
    res = bass_utils.run_bass_kernel_spmd(
        nc, in_maps, core_ids=list(range(cfg.NCORES)), trace=trace,
        tmpdir=tmpdir)
    if trace_out is not None:
        trace_out.append(res)
    outs = res.results

    out = np.empty((cfg.B, cfg.N, cfg.O), np.float32)
    for b in range(cfg.B):
        full = np.concatenate([outs[2 * b]["out"], outs[2 * b + 1]["out"]],
                              axis=0)
        out[b] = full[sigmas[b]]
    return out


def kernel(images, edges, pseudo, W1, root1, b1, W2, root2, b2):
    cfg = CFG()
    return run(cfg, images, edges, pseudo, W1, root1, b1,
               W2, root2, b2)



# revision 4
# speedup vs baseline: 1.3430x; 1.3430x over previous
"""Trainium2 Bass kernel for nn_MeshDownConv (2-layer SplineConv GNN).

Sharding: 8 cores = 4 meshes x 2 dst-halves. Host relabels nodes so each
128-node "window" has a near-uniform in-edge count, sorts edges by window,
and precomputes the 9 B-spline basis values per edge. Edges in a window are
split into two passes by source half (A: src < NHALF, B: src >= NHALF) so
dma_gather's int16 indices can address the table. Device pipeline per
window: dma_gather of source features (edge-partitioned, 256B rows), one
DVE tensor_tensor per pass builds u[e, (k,c)] = basis_k[e] * xj[e, c] via
step-0 broadcast APs, one DVE tensor_tensor builds the one-hot incidence
inc[e, n] = (dstrel[e] == n), then 18 PE matmuls contract the edge dim:
z[n, (k,c)] += inc^T @ u  (PSUM accumulation). Node-side: transpose z,
multiply by Wflat chunks, add root term + bias, relu. Halves exchange
node features with a 2-core AllGather between the layers.
"""
import sys

sys.path.insert(0, "/opt/trn_rl_repo")

import numpy as np

import concourse.bass as bass
import concourse.mybir as mybir
from concourse import bacc, tile, bass_utils

F32 = mybir.dt.float32
I16 = mybir.dt.int16
I32 = mybir.dt.int32


class CFG:
    C = 32            # in channels
    O = 32            # out channels
    KK = 9            # spline kernels

    @property
    def RL(self):
        # table row length: 256B granule for dma_gather
        return 64 if self.DT == F32 else 128
    NW = 196          # windows per half
    NCHA = 9          # chunks (of 128 edges) per window per pass
    GW = 4            # windows per gather group
    N = 50000         # real nodes per mesh
    E = 800000        # edges per mesh
    B = 4             # meshes
    NCORES = 8
    DT = mybir.dt.float16   # edge-side dtype (table/xj/u/inc/basis)
    UPOOL = 5               # u-build chunks offloaded to gpsimd (of NCH)
    INCPOOL = False         # build incidence on gpsimd (else DVE)

    @property
    def NCH(self):
        return 2 * self.NCHA

    @property
    def NHALF(self):
        return self.NW * 128

    @property
    def NN(self):
        return 2 * self.NHALF

    @property
    def NCHT(self):
        return self.NW * self.NCH

    @property
    def NSLOTA(self):
        # gather slots per half-core per pass
        return self.NW * self.NCHA * 128


def _np_dt(dt):
    return {F32: np.float32, mybir.dt.float16: np.float16}[dt]


# ----------------------------------------------------------------- host prep

def _quad_basis_np(t):
    return np.stack([0.5 * (1.0 - t) ** 2, -t * t + t + 0.5, 0.5 * t * t],
                    axis=-1)


def _balance_nodes(deg, nbins, cap_nodes=128):
    """Greedy: assign nodes (desc by degree) to the lightest non-full bin.
    Returns sigma: old -> new id (bin*128 + slot)."""
    import heapq
    n = deg.shape[0]
    order = np.argsort(-deg, kind="stable")
    heap = [(0, b) for b in range(nbins)]
    heapq.heapify(heap)
    counts = np.zeros(nbins, np.int64)
    sums = np.zeros(nbins, np.int64)
    sigma = np.empty(n, np.int64)
    for old in order:
        while True:
            s, b = heapq.heappop(heap)
            if counts[b] < cap_nodes:
                break
        sigma[old] = b * 128 + counts[b]
        counts[b] += 1
        sums[b] += deg[old]
        if counts[b] < cap_nodes:
            heapq.heappush(heap, (sums[b], b))
    return sigma, sums


def _host_prep_mesh(cfg, x, edge, pseudo):
    """Per-mesh host preprocessing.

    Returns (tab0 [NN, RL] f32, halves, sigma) where halves[h] =
    (IDXA [16, NSLOTA/16] i16, IDXB, ED [128, NCHT, 10] DT).
    """
    npdt = _np_dt(cfg.DT)
    src, dst = edge[0].astype(np.int64), edge[1].astype(np.int64)
    E = src.shape[0]

    B0 = _quad_basis_np(pseudo[:, 0].astype(np.float32))
    B1 = _quad_basis_np(pseudo[:, 1].astype(np.float32))
    basis = (B1[:, :, None] * B0[:, None, :]).reshape(E, cfg.KK)

    deg = np.bincount(dst, minlength=cfg.N)
    sigma, sums = _balance_nodes(deg, 2 * cfg.NW)

    new_dst = sigma[dst]
    new_src = sigma[src]

    tab0 = np.zeros((cfg.NN, cfg.RL), _np_dt(cfg.DT))
    tab0[sigma[np.arange(cfg.N)], :cfg.C] = x.astype(_np_dt(cfg.DT))

    # order edges by (window, src-half)
    win = new_dst // 128
    srchalf = (new_src >= cfg.NHALF).astype(np.int64)
    key = win * 2 + srchalf
    order = np.argsort(key, kind="stable")
    key_sorted = key[order]
    starts = np.searchsorted(key_sorted, np.arange(4 * cfg.NW + 1))
    capa = cfg.NCHA * 128
    halves = []
    for h in range(2):
        IDX = np.zeros((2, 16, cfg.NSLOTA // 16), np.int16)  # wrapped
        ED = np.zeros((128, cfg.NCHT, 1 + cfg.KK), np.float32)
        for wl in range(cfg.NW):
            w = h * cfg.NW + wl
            for p in range(2):  # pass A/B
                lo, hi = starts[2 * w + p], starts[2 * w + p + 1]
                eids = order[lo:hi]
                cnt = hi - lo
                assert cnt <= capa, (cnt, capa, w, p)
                slot = np.arange(cnt)
                ch = (p * cfg.NCHA + wl * cfg.NCH) + slot // 128
                lane = slot % 128
                ED[lane, ch, 0] = (new_dst[eids] - w * 128).astype(np.float32)
                ED[lane, ch, 1:] = basis[eids]
                gslot = (wl * cfg.NCHA + slot // 128) * 128 + lane
                sidx = new_src[eids] - p * cfg.NHALF
                IDX[p, gslot % 16, gslot // 16] = sidx.astype(np.int16)
        halves.append((np.tile(IDX[0], (8, 1)), np.tile(IDX[1], (8, 1)),
                       ED.astype(npdt)))
    return tab0, halves, sigma


# ------------------------------------------------------------- bass program

def _ap(t, offset, pattern):
    base = t if isinstance(t, bass.AP) else t[:]
    return bass.AP(base.tensor, base.offset + offset, pattern)


def _sap(t, offset, freedims, npart=None):
    """Slice of an SBUF/PSUM tile in the flat AP convention."""
    base = t if isinstance(t, bass.AP) else t[:]
    p = base.ap[0]
    part = [p[0], p[1] if npart is None else npart]
    return bass.AP(base.tensor, base.offset + offset, [part] + list(freedims))


def build_program(cfg, ncores, sim_single=False):
    nc = bacc.Bacc("TRN2", target_bir_lowering=False, debug=False,
                   num_devices=ncores, num_swdge_queues=4)
    DT = cfg.DT
    C, O, KK, RL = cfg.C, cfg.O, cfg.KK, cfg.RL
    NCH, NCHA, GW, NW = cfg.NCH, cfg.NCHA, cfg.GW, cfg.NW
    NG = NW // GW
    UD = KK * C   # 288
    NI16 = cfg.NSLOTA // 16

    tab0 = nc.dram_tensor("tab0", [cfg.NN, RL], DT, kind="ExternalInput")
    xown0 = nc.dram_tensor("xown0", [cfg.NHALF, RL], DT,
                           kind="ExternalInput")
    idxa = nc.dram_tensor("idxa", [128, NI16], I16, kind="ExternalInput")
    idxb = nc.dram_tensor("idxb", [128, NI16], I16, kind="ExternalInput")
    edd = nc.dram_tensor("edd", [128, cfg.NCHT, 1 + KK], DT,
                         kind="ExternalInput")
    wf1 = nc.dram_tensor("wf1", [96, 96], F32, kind="ExternalInput")
    wf2 = nc.dram_tensor("wf2", [96, 96], F32, kind="ExternalInput")
    rt1 = nc.dram_tensor("rt1", [C, O], DT, kind="ExternalInput")
    rt2 = nc.dram_tensor("rt2", [C, O], DT, kind="ExternalInput")
    bb1 = nc.dram_tensor("bb1", [O, 1], F32, kind="ExternalInput")
    bb2 = nc.dram_tensor("bb2", [O, 1], F32, kind="ExternalInput")
    outt = nc.dram_tensor("out", [cfg.NHALF, C], F32, kind="ExternalOutput")

    replica_groups = [[2 * i, 2 * i + 1] for i in range(ncores // 2)]

    with tile.TileContext(nc, num_cores=ncores) as tc:
        with tc.tile_pool(name="const", bufs=1) as cpool, \
             tc.tile_pool(name="dram", bufs=1, space="DRAM") as dpool, \
             tc.tile_pool(name="work", bufs=2) as wpool, \
             tc.tile_pool(name="psum", bufs=1, space="PSUM") as ppool:

            from concourse import library_config
            nc.gpsimd.load_library(library_config.mlp)

            # ---- constants
            iotai = cpool.tile([128, 128], I32, name="iotai")
            nc.gpsimd.iota(iotai[:], pattern=[[1, 128]], base=0,
                           channel_multiplier=0)
            iotaf = cpool.tile([128, 128], DT, name="iotaf")
            nc.vector.tensor_copy(iotaf[:], iotai[:])
            idximp = cpool.tile([128, 128], I32, name="idximp")
            nc.gpsimd.iota(idximp[:], pattern=[[1, 128]], base=0,
                           channel_multiplier=-1)
            identf = cpool.tile([128, 128], F32, name="identf")
            nc.vector.tensor_scalar(out=identf[:], in0=idximp[:],
                                    scalar1=0, scalar2=None,
                                    op0=mybir.AluOpType.is_equal)
            identd = identf
            if DT != F32:
                identd = cpool.tile([128, 128], DT, name="identd")
                nc.vector.tensor_copy(identd[:], identf[:])
            zrowa = cpool.tile([1, 128], DT, name="zrowa")
            nc.vector.memset(zrowa[:], 0)
            zrowb = cpool.tile([1, UD], DT, name="zrowb")
            nc.vector.memset(zrowb[:], 0)

            # index tables stay resident in SBUF
            sidxa = cpool.tile([128, NI16], I16, name="sidxa")
            nc.sync.dma_start(sidxa[:], idxa.ap())
            sidxb = cpool.tile([128, NI16], I16, name="sidxb")
            nc.sync.dma_start(sidxb[:], idxb.ap())

            wfs = [cpool.tile([96, 96], F32, name=f"wfs{i}") for i in range(2)]
            rts = [cpool.tile([C, O], DT, name=f"rts{i}") for i in range(2)]
            bbs = [cpool.tile([O, 1], F32, name=f"bbs{i}") for i in range(2)]
            for i, (wsrc, rsrc, bsrc) in enumerate(
                    [(wf1, rt1, bb1), (wf2, rt2, bb2)]):
                nc.sync.dma_start(wfs[i][:], wsrc[:])
                nc.sync.dma_start(rts[i][:], rsrc[:])
                nc.sync.dma_start(bbs[i][:], bsrc[:])

            # ---- internal dram
            town = dpool.tile([cfg.NHALF, RL], DT, name="town")
            tfull = dpool.tile([cfg.NN, RL], DT, name="tfull")

            def layer(tabfull, xown, wfsb, rtsb, bbsb, rows_out, last):
                for g in range(NG):
                    nsg = GW * NCHA * 128       # gather idxs per pass
                    xjt = [None, None]
                    for p, sidx in ((0, sidxa), (1, sidxb)):
                        xj = wpool.tile([128, GW * NCHA * RL], DT,
                                        name=f"xj{p}", bufs=2)
                        # custom-DMA SBUF APs use the flat convention:
                        # partition step = row length in elements
                        nc.gpsimd.dma_gather(
                            out_ap=_sap(xj, 0, [[RL, GW * NCHA],
                                                [1, RL]]),
                            in_ap=_ap(tabfull, p * cfg.NHALF * RL,
                                      [[RL, cfg.NHALF], [1, RL]]),
                            idxs_ap=_sap(sidx, g * nsg // 16,
                                         [[1, nsg // 16]]),
                            num_idxs=nsg,
                            num_idxs_reg=nsg,
                            elem_size=RL,
                            single_packet=False,
                            queue_num=(2 * g + p) % 4,
                        )
                        xjt[p] = xj
                    edt = wpool.tile([128, GW * NCH * (1 + KK)], DT,
                                     name="edt", bufs=2)
                    nc.sync.dma_start(
                        edt[:],
                        _ap(edd.ap(), g * GW * NCH * (1 + KK),
                            [[cfg.NCHT * (1 + KK), 128],
                             [1, GW * NCH * (1 + KK)]]))
                    xwing = wpool.tile([128, GW * C], DT, name="xwing",
                                       bufs=2)
                    nc.sync.dma_start(
                        xwing[:],
                        _ap(xown, g * GW * 128 * RL,
                            [[RL, 128], [128 * RL, GW], [1, C]]))
                    out_dt = F32 if last else DT
                    rowsg = wpool.tile([128, GW * C], out_dt, name="rowsg",
                                       bufs=2)

                    for wl in range(GW):
                        w = g * GW + wl
                        u = wpool.tile([128, NCH * UD], DT, name="u", bufs=2)
                        for p in range(2):
                            # give the tail of pass B's chunks to gpsimd
                            npool = min(cfg.UPOOL, NCHA) if p == 1 else 0
                            ndve = NCHA - npool
                            if ndve:
                                nc.vector.tensor_tensor(
                                    out=_sap(u, p * NCHA * UD,
                                             [[UD, ndve], [C, KK], [1, C]]),
                                    in0=_sap(xjt[p], wl * NCHA * RL,
                                             [[RL, ndve], [0, KK], [1, C]]),
                                    in1=_sap(edt,
                                             (wl * NCH + p * NCHA) * (1 + KK)
                                             + 1,
                                             [[1 + KK, ndve], [1, KK],
                                              [0, C]]),
                                    op=mybir.AluOpType.mult)
                            if npool:
                                nc.gpsimd.tensor_tensor(
                                    out=_sap(u, (p * NCHA + ndve) * UD,
                                             [[UD, npool], [C, KK], [1, C]]),
                                    in0=_sap(xjt[p],
                                             (wl * NCHA + ndve) * RL,
                                             [[RL, npool], [0, KK], [1, C]]),
                                    in1=_sap(edt,
                                             (wl * NCH + p * NCHA + ndve)
                                             * (1 + KK) + 1,
                                             [[1 + KK, npool], [1, KK],
                                              [0, C]]),
                                    op=mybir.AluOpType.mult)
                        inc = wpool.tile([128, NCH * 128], DT, name="inc",
                                         bufs=2)
                        inc_eng = nc.gpsimd if cfg.INCPOOL else nc.vector
                        inc_eng.tensor_tensor(
                            out=_sap(inc, 0, [[128, NCH], [1, 128]]),
                            in0=_sap(iotaf, 0, [[0, NCH], [1, 128]]),
                            in1=_sap(edt, wl * NCH * (1 + KK),
                                     [[1 + KK, NCH], [0, 128]]),
                            op=mybir.AluOpType.is_equal)

                        z = ppool.tile([128, UD], F32, name="z", bufs=2)
                        for c in range(NCH):
                            # every chunk matmul writes all of z densely, so
                            # no zero-init is needed
                            nc.tensor.matmul(
                                z[:],
                                _sap(inc, c * 128, [[1, 128]]),
                                _sap(u, c * UD, [[1, UD]]),
                                start=(c == 0), stop=(c == NCH - 1))

                        zsb = wpool.tile([128, UD], F32, name="zsb", bufs=2)
                        nc.scalar.copy(zsb[:], z[:])
                        zt = ppool.tile([96, 384], F32, name="zt", bufs=2)
                        for j in range(3):
                            nc.tensor.transpose(
                                _sap(zt, j * 128, [[1, 128]]),
                                _sap(zsb, j * 96, [[1, 96]]),
                                identf[:])
                        ztsb = wpool.tile([96, 384], F32, name="ztsb", bufs=2)
                        nc.scalar.copy(ztsb[:], zt[:])

                        agg = ppool.tile([O, 128], F32, name="agg", bufs=1)
                        for j in range(3):
                            nc.tensor.matmul(
                                agg[:],
                                _sap(wfsb, j * 32, [[1, 32]]),
                                _sap(ztsb, j * 128, [[1, 128]]),
                                start=(j == 0), stop=False)
                        xt = ppool.tile([C, 128], DT, name="xt", bufs=1)
                        nc.tensor.transpose(
                            xt[:],
                            _sap(xwing, wl * C, [[1, C]]),
                            identd[:])
                        xtsb = wpool.tile([C, 128], DT, name="xtsb", bufs=2)
                        nc.scalar.copy(xtsb[:], xt[:])
                        nc.tensor.matmul(agg[:], rtsb[:], xtsb[:],
                                         start=False, stop=True)
                        ht = wpool.tile([O, 128], out_dt, name="ht", bufs=2)
                        nc.scalar.activation(
                            ht[:], agg[:],
                            mybir.ActivationFunctionType.Relu,
                            bias=bbsb[:], scale=1.0)
                        rows = ppool.tile([128, O], out_dt, name="rows",
                                          bufs=1)
                        nc.tensor.transpose(
                            rows[:], ht[:],
                            _sap(identf if last else identd, 0,
                                 [[1, 32]], npart=32))
                        nc.scalar.copy(
                            _sap(rowsg, wl * C, [[1, C]]), rows[:])

                    if last:
                        nc.sync.dma_start(
                            _ap(rows_out, g * GW * 128 * C,
                                [[C, 128], [128 * C, GW], [1, C]]),
                            rowsg[:])
                    else:
                        nc.sync.dma_start(
                            _ap(rows_out, g * GW * 128 * RL,
                                [[RL, 128], [128 * RL, GW], [1, C]]),
                            rowsg[:])

            # layer 1
            layer(tab0.ap(), xown0.ap(), wfs[0], rts[0], bbs[0],
                  town[:], last=False)
            if sim_single:
                # collective-free stand-in for single-core timeline sim:
                # same bytes moved (NHALF rows in + out per core)
                nc.sync.dma_start(
                    _ap(tfull[:], 0, [[1, cfg.NHALF * RL]]),
                    _ap(town[:], 0, [[1, cfg.NHALF * RL]]))
                nc.sync.dma_start(
                    _ap(tfull[:], cfg.NHALF * RL, [[1, cfg.NHALF * RL]]),
                    _ap(town[:], 0, [[1, cfg.NHALF * RL]]))
            else:
                nc.gpsimd.collective_compute(
                    "AllGather", mybir.AluOpType.bypass,
                    replica_groups=replica_groups,
                    ins=[town[:]], outs=[tfull[:]])
            # layer 2
            layer(tfull[:], town[:], wfs[1], rts[1], bbs[1],
                  outt.ap(), last=True)

    nc.finalize()
    return nc


# ------------------------------------------------------------------- driver

_cache = {}


def _get_program(cfg):
    key = (cfg.NW, cfg.NCHA, cfg.GW, cfg.DT, cfg.NCORES,
           cfg.UPOOL, cfg.INCPOOL)
    if key not in _cache:
        _cache[key] = build_program(cfg, cfg.NCORES)
    return _cache[key]


def run(cfg, images, edges, pseudo, W1, root1, b1, W2, root2, b2,
        trace=False, trace_out=None, tmpdir=None):
    wf = []
    for W in (W1, W2):
        Wflat = np.asarray(W, np.float32).reshape(cfg.KK * cfg.C, cfg.O)
        wfl = np.zeros((96, 96), np.float32)
        for j in range(3):
            wfl[:, 32 * j:32 * j + 32] = Wflat[96 * j:96 * j + 96, :]
        wf.append(wfl)
    rts = [np.asarray(r, np.float32).astype(_np_dt(cfg.DT))
           for r in (root1, root2)]
    bbs = [np.asarray(b, np.float32).reshape(cfg.O, 1) for b in (b1, b2)]

    in_maps = []
    sigmas = []
    for b in range(cfg.B):
        tab0, halves, sigma = _host_prep_mesh(
            cfg, np.asarray(images[b], np.float32),
            np.asarray(edges[b]), np.asarray(pseudo[b], np.float32))
        sigmas.append(sigma)
        for h in range(2):
            IDXA, IDXB, ED = halves[h]
            in_maps.append({
                "tab0": tab0,
                "xown0": tab0[h * cfg.NHALF:(h + 1) * cfg.NHALF],
                "idxa": IDXA, "idxb": IDXB,
                "edd": ED,
                "wf1": wf[0], "wf2": wf[1],
                "rt1": rts[0], "rt2": rts[1],
                "bb1": bbs[0], "bb2": bbs[1],
            })

    nc = _get_program(cfg)
    res = bass_utils.run_bass_kernel_spmd(
        nc, in_maps, core_ids=list(range(cfg.NCORES)), trace=trace,
        tmpdir=tmpdir)
    if trace_out is not None:
        trace_out.append(res)
    outs = res.results

    out = np.empty((cfg.B, cfg.N, cfg.O), np.float32)
    for b in range(cfg.B):
        full = np.concatenate([outs[2 * b]["out"], outs[2 * b + 1]["out"]],
                              axis=0)
        out[b] = full[sigmas[b]]
    return out


def kernel(images, edges, pseudo, W1, root1, b1, W2, root2, b2):
    cfg = CFG()
    return run(cfg, images, edges, pseudo, W1, root1, b1,
               W2, root2, b2)



# revision 5
# speedup vs baseline: 1.7885x; 1.3317x over previous
"""Trainium2 Bass kernel for nn_MeshDownConv (2-layer SplineConv GNN).

Sharding: 8 cores = 4 meshes x 2 dst-halves. Host relabels nodes so each
128-node "window" has a near-uniform in-edge count, sorts edges by window,
and precomputes the 9 B-spline basis values per edge. Edges in a window are
split into two passes by source half (A: src < NHALF, B: src >= NHALF) so
dma_gather's int16 indices can address the table. Device pipeline per
window: dma_gather of source features (edge-partitioned, 256B rows), one
DVE tensor_tensor per pass builds u[e, (k,c)] = basis_k[e] * xj[e, c] via
step-0 broadcast APs, one DVE tensor_tensor builds the one-hot incidence
inc[e, n] = (dstrel[e] == n), then 18 PE matmuls contract the edge dim:
z[n, (k,c)] += inc^T @ u  (PSUM accumulation). Node-side: transpose z,
multiply by Wflat chunks, add root term + bias, relu. Halves exchange
node features with a 2-core AllGather between the layers.
"""
import sys

sys.path.insert(0, "/opt/trn_rl_repo")

import numpy as np

import concourse.bass as bass
import concourse.mybir as mybir
from concourse import bacc, tile, bass_utils

F32 = mybir.dt.float32
I16 = mybir.dt.int16
I32 = mybir.dt.int32


class CFG:
    C = 32            # in channels
    O = 32            # out channels
    KK = 9            # spline kernels

    @property
    def RL(self):
        # table row length: 256B granule for dma_gather
        return 64 if self.DT == F32 else 128
    NW = 196          # windows per half
    NCHA = 9          # chunks (of 128 edges) per window per pass
    GW = 4            # windows per gather group
    N = 50000         # real nodes per mesh
    E = 800000        # edges per mesh
    B = 4             # meshes
    NCORES = 8
    DT = mybir.dt.float16   # edge-side dtype (table/xj/u/inc/basis)
    UPOOL = 0               # u-build chunks offloaded to gpsimd (of NCH)
    INCPOOL = False         # build incidence on gpsimd (else DVE)

    @property
    def NCH(self):
        return 2 * self.NCHA

    @property
    def NHALF(self):
        return self.NW * 128

    @property
    def NN(self):
        return 2 * self.NHALF

    @property
    def NCHT(self):
        return self.NW * self.NCH

    @property
    def NSLOTA(self):
        # gather slots per half-core per pass
        return self.NW * self.NCHA * 128


def _np_dt(dt):
    return {F32: np.float32, mybir.dt.float16: np.float16}[dt]


# ----------------------------------------------------------------- host prep

def _quad_basis_np(t):
    return np.stack([0.5 * (1.0 - t) ** 2, -t * t + t + 0.5, 0.5 * t * t],
                    axis=-1)


def _balance_nodes(deg, nbins, cap_nodes=128):
    """Greedy: assign nodes (desc by degree) to the lightest non-full bin.
    Returns sigma: old -> new id (bin*128 + slot)."""
    import heapq
    n = deg.shape[0]
    order = np.argsort(-deg, kind="stable")
    heap = [(0, b) for b in range(nbins)]
    heapq.heapify(heap)
    counts = np.zeros(nbins, np.int64)
    sums = np.zeros(nbins, np.int64)
    sigma = np.empty(n, np.int64)
    for old in order:
        while True:
            s, b = heapq.heappop(heap)
            if counts[b] < cap_nodes:
                break
        sigma[old] = b * 128 + counts[b]
        counts[b] += 1
        sums[b] += deg[old]
        if counts[b] < cap_nodes:
            heapq.heappush(heap, (sums[b], b))
    return sigma, sums


def _host_prep_mesh(cfg, x, edge, pseudo):
    """Per-mesh host preprocessing.

    Returns (tab0 [NN, RL] f32, halves, sigma) where halves[h] =
    (IDXA [16, NSLOTA/16] i16, IDXB, ED [128, NCHT, 10] DT).
    """
    npdt = _np_dt(cfg.DT)
    src, dst = edge[0].astype(np.int64), edge[1].astype(np.int64)
    E = src.shape[0]

    B0 = _quad_basis_np(pseudo[:, 0].astype(np.float32))
    B1 = _quad_basis_np(pseudo[:, 1].astype(np.float32))
    basis = (B1[:, :, None] * B0[:, None, :]).reshape(E, cfg.KK)

    deg = np.bincount(dst, minlength=cfg.N)
    sigma, sums = _balance_nodes(deg, 2 * cfg.NW)

    new_dst = sigma[dst]
    new_src = sigma[src]

    tab0 = np.zeros((cfg.NN, cfg.RL), _np_dt(cfg.DT))
    tab0[sigma[np.arange(cfg.N)], :cfg.C] = x.astype(_np_dt(cfg.DT))

    # order edges by (window, src-half)
    win = new_dst // 128
    srchalf = (new_src >= cfg.NHALF).astype(np.int64)
    key = win * 2 + srchalf
    order = np.argsort(key, kind="stable")
    key_sorted = key[order]
    starts = np.searchsorted(key_sorted, np.arange(4 * cfg.NW + 1))
    capa = cfg.NCHA * 128
    halves = []
    for h in range(2):
        IDX = np.zeros((2, 16, cfg.NSLOTA // 16), np.int16)  # wrapped
        ED = np.zeros((128, cfg.NCHT, 1 + cfg.KK), np.float32)
        for wl in range(cfg.NW):
            w = h * cfg.NW + wl
            for p in range(2):  # pass A/B
                lo, hi = starts[2 * w + p], starts[2 * w + p + 1]
                eids = order[lo:hi]
                cnt = hi - lo
                assert cnt <= capa, (cnt, capa, w, p)
                slot = np.arange(cnt)
                ch = (p * cfg.NCHA + wl * cfg.NCH) + slot // 128
                lane = slot % 128
                ED[lane, ch, 0] = (new_dst[eids] - w * 128).astype(np.float32)
                ED[lane, ch, 1:] = basis[eids]
                gslot = (wl * cfg.NCHA + slot // 128) * 128 + lane
                sidx = new_src[eids] - p * cfg.NHALF
                IDX[p, gslot % 16, gslot // 16] = sidx.astype(np.int16)
        halves.append((np.tile(IDX[0], (8, 1)), np.tile(IDX[1], (8, 1)),
                       ED.astype(npdt)))
    return tab0, halves, sigma


# ------------------------------------------------------------- bass program

def _ap(t, offset, pattern):
    base = t if isinstance(t, bass.AP) else t[:]
    return bass.AP(base.tensor, base.offset + offset, pattern)


def _sap(t, offset, freedims, npart=None):
    """Slice of an SBUF/PSUM tile in the flat AP convention."""
    base = t if isinstance(t, bass.AP) else t[:]
    p = base.ap[0]
    part = [p[0], p[1] if npart is None else npart]
    return bass.AP(base.tensor, base.offset + offset, [part] + list(freedims))


def build_program(cfg, ncores, sim_single=False):
    nc = bacc.Bacc("TRN2", target_bir_lowering=False, debug=False,
                   num_devices=ncores, num_swdge_queues=4,
                   dynamic_dma_scratch_size=49152)
    DT = cfg.DT
    C, O, KK, RL = cfg.C, cfg.O, cfg.KK, cfg.RL
    NCH, NCHA, GW, NW = cfg.NCH, cfg.NCHA, cfg.GW, cfg.NW
    NG = NW // GW
    UD = KK * C   # 288
    NI16 = cfg.NSLOTA // 16

    tab0 = nc.dram_tensor("tab0", [cfg.NN, RL], DT, kind="ExternalInput")
    xown0 = nc.dram_tensor("xown0", [cfg.NHALF, RL], DT,
                           kind="ExternalInput")
    idxa = nc.dram_tensor("idxa", [128, NI16], I16, kind="ExternalInput")
    idxb = nc.dram_tensor("idxb", [128, NI16], I16, kind="ExternalInput")
    edd = nc.dram_tensor("edd", [128, cfg.NCHT, 1 + KK], DT,
                         kind="ExternalInput")
    wf1 = nc.dram_tensor("wf1", [96, 96], F32, kind="ExternalInput")
    wf2 = nc.dram_tensor("wf2", [96, 96], F32, kind="ExternalInput")
    rt1 = nc.dram_tensor("rt1", [C, O], DT, kind="ExternalInput")
    rt2 = nc.dram_tensor("rt2", [C, O], DT, kind="ExternalInput")
    bb1 = nc.dram_tensor("bb1", [O, 1], F32, kind="ExternalInput")
    bb2 = nc.dram_tensor("bb2", [O, 1], F32, kind="ExternalInput")
    outt = nc.dram_tensor("out", [cfg.NHALF, C], F32, kind="ExternalOutput")

    replica_groups = [[2 * i, 2 * i + 1] for i in range(ncores // 2)]

    with tile.TileContext(nc, num_cores=ncores) as tc:
        with tc.tile_pool(name="const", bufs=1) as cpool, \
             tc.tile_pool(name="dram", bufs=1, space="DRAM") as dpool, \
             tc.tile_pool(name="work", bufs=2) as wpool, \
             tc.tile_pool(name="psum", bufs=1, space="PSUM") as ppool:

            from concourse import library_config
            nc.gpsimd.load_library(library_config.mlp)

            # ---- constants
            iotai = cpool.tile([128, 128], I32, name="iotai")
            nc.gpsimd.iota(iotai[:], pattern=[[1, 128]], base=0,
                           channel_multiplier=0)
            iotaf = cpool.tile([128, 128], DT, name="iotaf")
            nc.vector.tensor_copy(iotaf[:], iotai[:])
            idximp = cpool.tile([128, 128], I32, name="idximp")
            nc.gpsimd.iota(idximp[:], pattern=[[1, 128]], base=0,
                           channel_multiplier=-1)
            identf = cpool.tile([128, 128], F32, name="identf")
            nc.vector.tensor_scalar(out=identf[:], in0=idximp[:],
                                    scalar1=0, scalar2=None,
                                    op0=mybir.AluOpType.is_equal)
            identd = identf
            if DT != F32:
                identd = cpool.tile([128, 128], DT, name="identd")
                nc.vector.tensor_copy(identd[:], identf[:])
            zrowa = cpool.tile([1, 128], DT, name="zrowa")
            nc.vector.memset(zrowa[:], 0)
            zrowb = cpool.tile([1, UD], DT, name="zrowb")
            nc.vector.memset(zrowb[:], 0)

            # index tables stay resident in SBUF
            sidxa = cpool.tile([128, NI16], I16, name="sidxa")
            nc.sync.dma_start(sidxa[:], idxa.ap())
            sidxb = cpool.tile([128, NI16], I16, name="sidxb")
            nc.sync.dma_start(sidxb[:], idxb.ap())

            wfs = [cpool.tile([96, 96], F32, name=f"wfs{i}") for i in range(2)]
            rts = [cpool.tile([C, O], DT, name=f"rts{i}") for i in range(2)]
            bbs = [cpool.tile([O, 1], F32, name=f"bbs{i}") for i in range(2)]
            for i, (wsrc, rsrc, bsrc) in enumerate(
                    [(wf1, rt1, bb1), (wf2, rt2, bb2)]):
                nc.sync.dma_start(wfs[i][:], wsrc[:])
                nc.sync.dma_start(rts[i][:], rsrc[:])
                nc.sync.dma_start(bbs[i][:], bsrc[:])

            # ---- internal dram
            town = dpool.tile([cfg.NHALF, RL], DT, name="town")
            tfull = dpool.tile([cfg.NN, RL], DT, name="tfull")

            def layer(tabfull, xown, wfsb, rtsb, bbsb, rows_out, last):
                for g in range(NG):
                    nsg = GW * NCHA * 128       # gather idxs per pass
                    xjt = [None, None]
                    for p, sidx in ((0, sidxa), (1, sidxb)):
                        xj = wpool.tile([128, GW * NCHA * RL], DT,
                                        name=f"xj{p}", bufs=2)
                        # custom-DMA SBUF APs use the flat convention:
                        # partition step = row length in elements
                        nc.gpsimd.dma_gather(
                            out_ap=_sap(xj, 0, [[RL, GW * NCHA],
                                                [1, RL]]),
                            in_ap=_ap(tabfull, p * cfg.NHALF * RL,
                                      [[RL, cfg.NHALF], [1, RL]]),
                            idxs_ap=_sap(sidx, g * nsg // 16,
                                         [[1, nsg // 16]]),
                            num_idxs=nsg,
                            num_idxs_reg=nsg,
                            elem_size=RL,
                            single_packet=False,
                            queue_num=(2 * g + p) % 4,
                        )
                        xjt[p] = xj
                    edt = wpool.tile([128, GW * NCH * (1 + KK)], DT,
                                     name="edt", bufs=2)
                    nc.sync.dma_start(
                        edt[:],
                        _ap(edd.ap(), g * GW * NCH * (1 + KK),
                            [[cfg.NCHT * (1 + KK), 128],
                             [1, GW * NCH * (1 + KK)]]))
                    xwing = wpool.tile([128, GW * C], DT, name="xwing",
                                       bufs=2)
                    nc.sync.dma_start(
                        xwing[:],
                        _ap(xown, g * GW * 128 * RL,
                            [[RL, 128], [128 * RL, GW], [1, C]]))
                    out_dt = F32 if last else DT
                    rowsg = wpool.tile([128, GW * C], out_dt, name="rowsg",
                                       bufs=2)

                    for wl in range(GW):
                        w = g * GW + wl
                        u = wpool.tile([128, NCH * UD], DT, name="u", bufs=2)
                        for p in range(2):
                            # give the tail of pass B's chunks to gpsimd
                            npool = min(cfg.UPOOL, NCHA) if p == 1 else 0
                            ndve = NCHA - npool
                            if ndve:
                                nc.vector.tensor_tensor(
                                    out=_sap(u, p * NCHA * UD,
                                             [[UD, ndve], [C, KK], [1, C]]),
                                    in0=_sap(xjt[p], wl * NCHA * RL,
                                             [[RL, ndve], [0, KK], [1, C]]),
                                    in1=_sap(edt,
                                             (wl * NCH + p * NCHA) * (1 + KK)
                                             + 1,
                                             [[1 + KK, ndve], [1, KK],
                                              [0, C]]),
                                    op=mybir.AluOpType.mult)
                            if npool:
                                nc.gpsimd.tensor_tensor(
                                    out=_sap(u, (p * NCHA + ndve) * UD,
                                             [[UD, npool], [C, KK], [1, C]]),
                                    in0=_sap(xjt[p],
                                             (wl * NCHA + ndve) * RL,
                                             [[RL, npool], [0, KK], [1, C]]),
                                    in1=_sap(edt,
                                             (wl * NCH + p * NCHA + ndve)
                                             * (1 + KK) + 1,
                                             [[1 + KK, npool], [1, KK],
                                              [0, C]]),
                                    op=mybir.AluOpType.mult)
                        inc = wpool.tile([128, NCH * 128], DT, name="inc",
                                         bufs=2)
                        inc_eng = nc.gpsimd if cfg.INCPOOL else nc.vector
                        inc_eng.tensor_tensor(
                            out=_sap(inc, 0, [[128, NCH], [1, 128]]),
                            in0=_sap(iotaf, 0, [[0, NCH], [1, 128]]),
                            in1=_sap(edt, wl * NCH * (1 + KK),
                                     [[1 + KK, NCH], [0, 128]]),
                            op=mybir.AluOpType.is_equal)

                        z = ppool.tile([128, UD], F32, name="z", bufs=2)
                        for c in range(NCH):
                            # every chunk matmul writes all of z densely, so
                            # no zero-init is needed
                            nc.tensor.matmul(
                                z[:],
                                _sap(inc, c * 128, [[1, 128]]),
                                _sap(u, c * UD, [[1, UD]]),
                                start=(c == 0), stop=(c == NCH - 1))

                        zsb = wpool.tile([128, UD], F32, name="zsb", bufs=2)
                        nc.scalar.copy(zsb[:], z[:])
                        zt = ppool.tile([96, 384], F32, name="zt", bufs=2)
                        for j in range(3):
                            nc.tensor.transpose(
                                _sap(zt, j * 128, [[1, 128]]),
                                _sap(zsb, j * 96, [[1, 96]]),
                                identf[:])
                        ztsb = wpool.tile([96, 384], F32, name="ztsb", bufs=2)
                        nc.scalar.copy(ztsb[:], zt[:])

                        agg = ppool.tile([O, 128], F32, name="agg", bufs=1)
                        for j in range(3):
                            nc.tensor.matmul(
                                agg[:],
                                _sap(wfsb, j * 32, [[1, 32]]),
                                _sap(ztsb, j * 128, [[1, 128]]),
                                start=(j == 0), stop=False)
                        xt = ppool.tile([C, 128], DT, name="xt", bufs=1)
                        nc.tensor.transpose(
                            xt[:],
                            _sap(xwing, wl * C, [[1, C]]),
                            identd[:])
                        xtsb = wpool.tile([C, 128], DT, name="xtsb", bufs=2)
                        nc.scalar.copy(xtsb[:], xt[:])
                        nc.tensor.matmul(agg[:], rtsb[:], xtsb[:],
                                         start=False, stop=True)
                        ht = wpool.tile([O, 128], out_dt, name="ht", bufs=2)
                        nc.scalar.activation(
                            ht[:], agg[:],
                            mybir.ActivationFunctionType.Relu,
                            bias=bbsb[:], scale=1.0)
                        rows = ppool.tile([128, O], out_dt, name="rows",
                                          bufs=1)
                        nc.tensor.transpose(
                            rows[:], ht[:],
                            _sap(identf if last else identd, 0,
                                 [[1, 32]], npart=32))
                        nc.scalar.copy(
                            _sap(rowsg, wl * C, [[1, C]]), rows[:])

                    if last:
                        nc.sync.dma_start(
                            _ap(rows_out, g * GW * 128 * C,
                                [[C, 128], [128 * C, GW], [1, C]]),
                            rowsg[:])
                    else:
                        nc.sync.dma_start(
                            _ap(rows_out, g * GW * 128 * RL,
                                [[RL, 128], [128 * RL, GW], [1, C]]),
                            rowsg[:])

            # layer 1
            layer(tab0.ap(), xown0.ap(), wfs[0], rts[0], bbs[0],
                  town[:], last=False)
            if sim_single:
                # collective-free stand-in for single-core timeline sim:
                # same bytes moved (NHALF rows in + out per core)
                nc.sync.dma_start(
                    _ap(tfull[:], 0, [[1, cfg.NHALF * RL]]),
                    _ap(town[:], 0, [[1, cfg.NHALF * RL]]))
                nc.sync.dma_start(
                    _ap(tfull[:], cfg.NHALF * RL, [[1, cfg.NHALF * RL]]),
                    _ap(town[:], 0, [[1, cfg.NHALF * RL]]))
            else:
                nc.gpsimd.collective_compute(
                    "AllGather", mybir.AluOpType.bypass,
                    replica_groups=replica_groups,
                    ins=[town[:]], outs=[tfull[:]])
            # layer 2
            layer(tfull[:], town[:], wfs[1], rts[1], bbs[1],
                  outt.ap(), last=True)

    nc.finalize()
    return nc


# ------------------------------------------------------------------- driver

_cache = {}


def _get_program(cfg):
    key = (cfg.NW, cfg.NCHA, cfg.GW, cfg.DT, cfg.NCORES,
           cfg.UPOOL, cfg.INCPOOL)
    if key not in _cache:
        _cache[key] = build_program(cfg, cfg.NCORES)
    return _cache[key]


def run(cfg, images, edges, pseudo, W1, root1, b1, W2, root2, b2,
        trace=False, trace_out=None, tmpdir=None):
    wf = []
    for W in (W1, W2):
        Wflat = np.asarray(W, np.float32).reshape(cfg.KK * cfg.C, cfg.O)
        wfl = np.zeros((96, 96), np.float32)
        for j in range(3):
            wfl[:, 32 * j:32 * j + 32] = Wflat[96 * j:96 * j + 96, :]
        wf.append(wfl)
    rts = [np.asarray(r, np.float32).astype(_np_dt(cfg.DT))
           for r in (root1, root2)]
    bbs = [np.asarray(b, np.float32).reshape(cfg.O, 1) for b in (b1, b2)]

    in_maps = []
    sigmas = []
    for b in range(cfg.B):
        tab0, halves, sigma = _host_prep_mesh(
            cfg, np.asarray(images[b], np.float32),
            np.asarray(edges[b]), np.asarray(pseudo[b], np.float32))
        sigmas.append(sigma)
        for h in range(2):
            IDXA, IDXB, ED = halves[h]
            in_maps.append({
                "tab0": tab0,
                "xown0": tab0[h * cfg.NHALF:(h + 1) * cfg.NHALF],
                "idxa": IDXA, "idxb": IDXB,
                "edd": ED,
                "wf1": wf[0], "wf2": wf[1],
                "rt1": rts[0], "rt2": rts[1],
                "bb1": bbs[0], "bb2": bbs[1],
            })

    nc = _get_program(cfg)
    res = bass_utils.run_bass_kernel_spmd(
        nc, in_maps, core_ids=list(range(cfg.NCORES)), trace=trace,
        tmpdir=tmpdir)
    if trace_out is not None:
        trace_out.append(res)
    outs = res.results

    out = np.empty((cfg.B, cfg.N, cfg.O), np.float32)
    for b in range(cfg.B):
        full = np.concatenate([outs[2 * b]["out"], outs[2 * b + 1]["out"]],
                              axis=0)
        out[b] = full[sigmas[b]]
    return out


def kernel(images, edges, pseudo, W1, root1, b1, W2, root2, b2):
    cfg = CFG()
    return run(cfg, images, edges, pseudo, W1, root1, b1,
               W2, root2, b2)



# revision 6
# speedup vs baseline: 1.8718x; 1.0466x over previous
"""Trainium2 Bass kernel for nn_MeshDownConv (2-layer SplineConv GNN).

Sharding: 8 cores = 4 meshes x 2 dst-halves. Host relabels nodes so each
128-node "window" has a near-uniform in-edge count, sorts edges by window,
and precomputes the 9 B-spline basis values per edge. Edges in a window are
split into two passes by source half (A: src < NHALF, B: src >= NHALF) so
dma_gather's int16 indices can address the table. Device pipeline per
window: dma_gather of source features (edge-partitioned, 256B rows), one
DVE tensor_tensor per pass builds u[e, (k,c)] = basis_k[e] * xj[e, c] via
step-0 broadcast APs, one DVE tensor_tensor builds the one-hot incidence
inc[e, n] = (dstrel[e] == n), then 18 PE matmuls contract the edge dim:
z[n, (k,c)] += inc^T @ u  (PSUM accumulation). Node-side: transpose z,
multiply by Wflat chunks, add root term + bias, relu. Halves exchange
node features with a 2-core AllGather between the layers.
"""
import sys

sys.path.insert(0, "/opt/trn_rl_repo")

import numpy as np

import concourse.bass as bass
import concourse.mybir as mybir
from concourse import bacc, tile, bass_utils

F32 = mybir.dt.float32
I16 = mybir.dt.int16
I32 = mybir.dt.int32


class CFG:
    C = 32            # in channels
    O = 32            # out channels
    KK = 9            # spline kernels

    @property
    def RL(self):
        # table row length: 256B granule for dma_gather
        return 64 if self.DT == F32 else 128
    NW = 196          # windows per half
    NCHA = 9          # chunks (of 128 edges) per window per pass
    GW = 4            # windows per gather group
    N = 50000         # real nodes per mesh
    E = 800000        # edges per mesh
    B = 4             # meshes
    NCORES = 8
    DT = mybir.dt.float16   # edge-side dtype (table/xj/u/inc/basis)
    UPOOL = 0               # u-build chunks offloaded to gpsimd (of NCH)
    INCPOOL = False         # build incidence on gpsimd (else DVE)

    @property
    def NCH(self):
        return 2 * self.NCHA

    @property
    def NHALF(self):
        return self.NW * 128

    @property
    def NN(self):
        return 2 * self.NHALF

    @property
    def NCHT(self):
        return self.NW * self.NCH

    @property
    def NSLOTA(self):
        # gather slots per half-core per pass
        return self.NW * self.NCHA * 128


def _np_dt(dt):
    return {F32: np.float32, mybir.dt.float16: np.float16}[dt]


# ----------------------------------------------------------------- host prep

def _quad_basis_np(t):
    return np.stack([0.5 * (1.0 - t) ** 2, -t * t + t + 0.5, 0.5 * t * t],
                    axis=-1)


def _balance_nodes(deg, nbins, cap_nodes=128):
    """Greedy: assign nodes (desc by degree) to the lightest non-full bin.
    Returns sigma: old -> new id (bin*128 + slot)."""
    import heapq
    n = deg.shape[0]
    order = np.argsort(-deg, kind="stable")
    heap = [(0, b) for b in range(nbins)]
    heapq.heapify(heap)
    counts = np.zeros(nbins, np.int64)
    sums = np.zeros(nbins, np.int64)
    sigma = np.empty(n, np.int64)
    for old in order:
        while True:
            s, b = heapq.heappop(heap)
            if counts[b] < cap_nodes:
                break
        sigma[old] = b * 128 + counts[b]
        counts[b] += 1
        sums[b] += deg[old]
        if counts[b] < cap_nodes:
            heapq.heappush(heap, (sums[b], b))
    return sigma, sums


def _host_prep_mesh(cfg, x, edge, pseudo):
    """Per-mesh host preprocessing.

    Returns (tab0 [NN, RL] f32, halves, sigma) where halves[h] =
    (IDXA [16, NSLOTA/16] i16, IDXB, ED [128, NCHT, 10] DT).
    """
    npdt = _np_dt(cfg.DT)
    src, dst = edge[0].astype(np.int64), edge[1].astype(np.int64)
    E = src.shape[0]

    B0 = _quad_basis_np(pseudo[:, 0].astype(np.float32))
    B1 = _quad_basis_np(pseudo[:, 1].astype(np.float32))
    basis = (B1[:, :, None] * B0[:, None, :]).reshape(E, cfg.KK)

    deg = np.bincount(dst, minlength=cfg.N)
    sigma, sums = _balance_nodes(deg, 2 * cfg.NW)

    new_dst = sigma[dst]
    new_src = sigma[src]

    tab0 = np.zeros((cfg.NN, cfg.RL), _np_dt(cfg.DT))
    tab0[sigma[np.arange(cfg.N)], :cfg.C] = x.astype(_np_dt(cfg.DT))

    # order edges by (window, src-half)
    win = new_dst // 128
    srchalf = (new_src >= cfg.NHALF).astype(np.int64)
    key = win * 2 + srchalf
    order = np.argsort(key, kind="stable")
    key_sorted = key[order]
    starts = np.searchsorted(key_sorted, np.arange(4 * cfg.NW + 1))
    capa = cfg.NCHA * 128
    halves = []
    for h in range(2):
        IDX = np.zeros((2, 16, cfg.NSLOTA // 16), np.int16)  # wrapped
        ED = np.zeros((128, cfg.NCHT, 1 + cfg.KK), np.float32)
        for wl in range(cfg.NW):
            w = h * cfg.NW + wl
            for p in range(2):  # pass A/B
                lo, hi = starts[2 * w + p], starts[2 * w + p + 1]
                eids = order[lo:hi]
                cnt = hi - lo
                assert cnt <= capa, (cnt, capa, w, p)
                slot = np.arange(cnt)
                ch = (p * cfg.NCHA + wl * cfg.NCH) + slot // 128
                lane = slot % 128
                ED[lane, ch, 0] = (new_dst[eids] - w * 128).astype(np.float32)
                ED[lane, ch, 1:] = basis[eids]
                gslot = (wl * cfg.NCHA + slot // 128) * 128 + lane
                sidx = new_src[eids] - p * cfg.NHALF
                IDX[p, gslot % 16, gslot // 16] = sidx.astype(np.int16)
        halves.append((np.tile(IDX[0], (8, 1)), np.tile(IDX[1], (8, 1)),
                       ED.astype(npdt)))
    return tab0, halves, sigma


# ------------------------------------------------------------- bass program

def _ap(t, offset, pattern):
    base = t if isinstance(t, bass.AP) else t[:]
    return bass.AP(base.tensor, base.offset + offset, pattern)


def _sap(t, offset, freedims, npart=None):
    """Slice of an SBUF/PSUM tile in the flat AP convention."""
    base = t if isinstance(t, bass.AP) else t[:]
    p = base.ap[0]
    part = [p[0], p[1] if npart is None else npart]
    return bass.AP(base.tensor, base.offset + offset, [part] + list(freedims))


def build_program(cfg, ncores, sim_single=False):
    nc = bacc.Bacc("TRN2", target_bir_lowering=False, debug=False,
                   num_devices=ncores, num_swdge_queues=4,
                   dynamic_dma_scratch_size=49152)
    DT = cfg.DT
    C, O, KK, RL = cfg.C, cfg.O, cfg.KK, cfg.RL
    NCH, NCHA, GW, NW = cfg.NCH, cfg.NCHA, cfg.GW, cfg.NW
    NG = NW // GW
    UD = KK * C   # 288
    NI16 = cfg.NSLOTA // 16

    tab0 = nc.dram_tensor("tab0", [cfg.NN, RL], DT, kind="ExternalInput")
    xown0 = nc.dram_tensor("xown0", [cfg.NHALF, RL], DT,
                           kind="ExternalInput")
    idxa = nc.dram_tensor("idxa", [128, NI16], I16, kind="ExternalInput")
    idxb = nc.dram_tensor("idxb", [128, NI16], I16, kind="ExternalInput")
    edd = nc.dram_tensor("edd", [128, cfg.NCHT, 1 + KK], DT,
                         kind="ExternalInput")
    wf1 = nc.dram_tensor("wf1", [96, 96], F32, kind="ExternalInput")
    wf2 = nc.dram_tensor("wf2", [96, 96], F32, kind="ExternalInput")
    rt1 = nc.dram_tensor("rt1", [C, O], DT, kind="ExternalInput")
    rt2 = nc.dram_tensor("rt2", [C, O], DT, kind="ExternalInput")
    bb1 = nc.dram_tensor("bb1", [O, 1], F32, kind="ExternalInput")
    bb2 = nc.dram_tensor("bb2", [O, 1], F32, kind="ExternalInput")
    outt = nc.dram_tensor("out", [cfg.NHALF, C], F32, kind="ExternalOutput")

    replica_groups = [[2 * i, 2 * i + 1] for i in range(ncores // 2)]

    with tile.TileContext(nc, num_cores=ncores) as tc:
        with tc.tile_pool(name="const", bufs=1) as cpool, \
             tc.tile_pool(name="dram", bufs=1, space="DRAM") as dpool, \
             tc.tile_pool(name="work", bufs=2) as wpool, \
             tc.tile_pool(name="psum", bufs=1, space="PSUM") as ppool:

            from concourse import library_config
            nc.gpsimd.load_library(library_config.mlp)

            # ---- constants
            iotai = cpool.tile([128, 128], I32, name="iotai")
            nc.gpsimd.iota(iotai[:], pattern=[[1, 128]], base=0,
                           channel_multiplier=0)
            iotaf = cpool.tile([128, 128], DT, name="iotaf")
            nc.vector.tensor_copy(iotaf[:], iotai[:])
            idximp = cpool.tile([128, 128], I32, name="idximp")
            nc.gpsimd.iota(idximp[:], pattern=[[1, 128]], base=0,
                           channel_multiplier=-1)
            identf = cpool.tile([128, 128], F32, name="identf")
            nc.vector.tensor_scalar(out=identf[:], in0=idximp[:],
                                    scalar1=0, scalar2=None,
                                    op0=mybir.AluOpType.is_equal)
            identd = identf
            if DT != F32:
                identd = cpool.tile([128, 128], DT, name="identd")
                nc.vector.tensor_copy(identd[:], identf[:])
            zrowa = cpool.tile([1, 128], DT, name="zrowa")
            nc.vector.memset(zrowa[:], 0)
            zrowb = cpool.tile([1, UD], DT, name="zrowb")
            nc.vector.memset(zrowb[:], 0)

            # index tables stay resident in SBUF
            sidxa = cpool.tile([128, NI16], I16, name="sidxa")
            nc.sync.dma_start(sidxa[:], idxa.ap())
            sidxb = cpool.tile([128, NI16], I16, name="sidxb")
            nc.sync.dma_start(sidxb[:], idxb.ap())

            wfs = [cpool.tile([96, 96], F32, name=f"wfs{i}") for i in range(2)]
            rts = [cpool.tile([C, O], DT, name=f"rts{i}") for i in range(2)]
            bbs = [cpool.tile([O, 1], F32, name=f"bbs{i}") for i in range(2)]
            for i, (wsrc, rsrc, bsrc) in enumerate(
                    [(wf1, rt1, bb1), (wf2, rt2, bb2)]):
                nc.sync.dma_start(wfs[i][:], wsrc[:])
                nc.sync.dma_start(rts[i][:], rsrc[:])
                nc.sync.dma_start(bbs[i][:], bsrc[:])

            # ---- internal dram
            town = dpool.tile([cfg.NHALF, RL], DT, name="town")
            tfull = dpool.tile([cfg.NN, RL], DT, name="tfull")

            def layer(tabfull, xown, wfsb, rtsb, bbsb, rows_out, last):
                for g in range(NG):
                    nsg = GW * NCHA * 128       # gather idxs per pass
                    xjt = [None, None]
                    for p, sidx in ((0, sidxa), (1, sidxb)):
                        xj = wpool.tile([128, GW * NCHA * RL], DT,
                                        name=f"xj{p}", bufs=2)
                        # custom-DMA SBUF APs use the flat convention:
                        # partition step = row length in elements.
                        # Split each gather in two: 146-desc halves fit the
                        # SWDGE ring next to an in-flight predecessor, so the
                        # instruction returns without waiting for the drain.
                        nh = GW // 2 * NCHA
                        for hf in range(2):
                            nc.gpsimd.dma_gather(
                                out_ap=_sap(xj, hf * nh * RL,
                                            [[RL, nh], [1, RL]]),
                                in_ap=_ap(tabfull, p * cfg.NHALF * RL,
                                          [[RL, cfg.NHALF], [1, RL]]),
                                idxs_ap=_sap(sidx,
                                             g * nsg // 16 + hf * nsg // 32,
                                             [[1, nsg // 32]]),
                                num_idxs=nsg // 2,
                                num_idxs_reg=nsg // 2,
                                elem_size=RL,
                                single_packet=False,
                                queue_num=(2 * p + hf) % 4,
                            )
                        xjt[p] = xj
                    edt = wpool.tile([128, GW * NCH * (1 + KK)], DT,
                                     name="edt", bufs=2)
                    nc.sync.dma_start(
                        edt[:],
                        _ap(edd.ap(), g * GW * NCH * (1 + KK),
                            [[cfg.NCHT * (1 + KK), 128],
                             [1, GW * NCH * (1 + KK)]]))
                    xwing = wpool.tile([128, GW * C], DT, name="xwing",
                                       bufs=2)
                    nc.sync.dma_start(
                        xwing[:],
                        _ap(xown, g * GW * 128 * RL,
                            [[RL, 128], [128 * RL, GW], [1, C]]))
                    out_dt = F32 if last else DT
                    rowsg = wpool.tile([128, GW * C], out_dt, name="rowsg",
                                       bufs=2)

                    for wl in range(GW):
                        w = g * GW + wl
                        u = wpool.tile([128, NCH * UD], DT, name="u", bufs=2)
                        for p in range(2):
                            # give the tail of pass B's chunks to gpsimd
                            npool = min(cfg.UPOOL, NCHA) if p == 1 else 0
                            ndve = NCHA - npool
                            if ndve:
                                nc.vector.tensor_tensor(
                                    out=_sap(u, p * NCHA * UD,
                                             [[UD, ndve], [C, KK], [1, C]]),
                                    in0=_sap(xjt[p], wl * NCHA * RL,
                                             [[RL, ndve], [0, KK], [1, C]]),
                                    in1=_sap(edt,
                                             (wl * NCH + p * NCHA) * (1 + KK)
                                             + 1,
                                             [[1 + KK, ndve], [1, KK],
                                              [0, C]]),
                                    op=mybir.AluOpType.mult)
                            if npool:
                                nc.gpsimd.tensor_tensor(
                                    out=_sap(u, (p * NCHA + ndve) * UD,
                                             [[UD, npool], [C, KK], [1, C]]),
                                    in0=_sap(xjt[p],
                                             (wl * NCHA + ndve) * RL,
                                             [[RL, npool], [0, KK], [1, C]]),
                                    in1=_sap(edt,
                                             (wl * NCH + p * NCHA + ndve)
                                             * (1 + KK) + 1,
                                             [[1 + KK, npool], [1, KK],
                                              [0, C]]),
                                    op=mybir.AluOpType.mult)
                        inc = wpool.tile([128, NCH * 128], DT, name="inc",
                                         bufs=2)
                        inc_eng = nc.gpsimd if cfg.INCPOOL else nc.vector
                        inc_eng.tensor_tensor(
                            out=_sap(inc, 0, [[128, NCH], [1, 128]]),
                            in0=_sap(iotaf, 0, [[0, NCH], [1, 128]]),
                            in1=_sap(edt, wl * NCH * (1 + KK),
                                     [[1 + KK, NCH], [0, 128]]),
                            op=mybir.AluOpType.is_equal)

                        z = ppool.tile([128, UD], F32, name="z", bufs=2)
                        for c in range(NCH):
                            # every chunk matmul writes all of z densely, so
                            # no zero-init is needed
                            nc.tensor.matmul(
                                z[:],
                                _sap(inc, c * 128, [[1, 128]]),
                                _sap(u, c * UD, [[1, UD]]),
                                start=(c == 0), stop=(c == NCH - 1))

                        zsb = wpool.tile([128, UD], F32, name="zsb", bufs=2)
                        nc.scalar.copy(zsb[:], z[:])
                        zt = ppool.tile([96, 384], F32, name="zt", bufs=2)
                        for j in range(3):
                            nc.tensor.transpose(
                                _sap(zt, j * 128, [[1, 128]]),
                                _sap(zsb, j * 96, [[1, 96]]),
                                identf[:])
                        ztsb = wpool.tile([96, 384], F32, name="ztsb", bufs=2)
                        nc.scalar.copy(ztsb[:], zt[:])

                        agg = ppool.tile([O, 128], F32, name="agg", bufs=1)
                        for j in range(3):
                            nc.tensor.matmul(
                                agg[:],
                                _sap(wfsb, j * 32, [[1, 32]]),
                                _sap(ztsb, j * 128, [[1, 128]]),
                                start=(j == 0), stop=False)
                        xt = ppool.tile([C, 128], DT, name="xt", bufs=1)
                        nc.tensor.transpose(
                            xt[:],
                            _sap(xwing, wl * C, [[1, C]]),
                            identd[:])
                        xtsb = wpool.tile([C, 128], DT, name="xtsb", bufs=2)
                        nc.scalar.copy(xtsb[:], xt[:])
                        nc.tensor.matmul(agg[:], rtsb[:], xtsb[:],
                                         start=False, stop=True)
                        ht = wpool.tile([O, 128], out_dt, name="ht", bufs=2)
                        nc.scalar.activation(
                            ht[:], agg[:],
                            mybir.ActivationFunctionType.Relu,
                            bias=bbsb[:], scale=1.0)
                        rows = ppool.tile([128, O], out_dt, name="rows",
                                          bufs=1)
                        nc.tensor.transpose(
                            rows[:], ht[:],
                            _sap(identf if last else identd, 0,
                                 [[1, 32]], npart=32))
                        nc.scalar.copy(
                            _sap(rowsg, wl * C, [[1, C]]), rows[:])

                    if last:
                        nc.sync.dma_start(
                            _ap(rows_out, g * GW * 128 * C,
                                [[C, 128], [128 * C, GW], [1, C]]),
                            rowsg[:])
                    else:
                        nc.sync.dma_start(
                            _ap(rows_out, g * GW * 128 * RL,
                                [[RL, 128], [128 * RL, GW], [1, C]]),
                            rowsg[:])

            # layer 1
            layer(tab0.ap(), xown0.ap(), wfs[0], rts[0], bbs[0],
                  town[:], last=False)
            if sim_single:
                # collective-free stand-in for single-core timeline sim:
                # same bytes moved (NHALF rows in + out per core)
                nc.sync.dma_start(
                    _ap(tfull[:], 0, [[1, cfg.NHALF * RL]]),
                    _ap(town[:], 0, [[1, cfg.NHALF * RL]]))
                nc.sync.dma_start(
                    _ap(tfull[:], cfg.NHALF * RL, [[1, cfg.NHALF * RL]]),
                    _ap(town[:], 0, [[1, cfg.NHALF * RL]]))
            else:
                nc.gpsimd.collective_compute(
                    "AllGather", mybir.AluOpType.bypass,
                    replica_groups=replica_groups,
                    ins=[town[:]], outs=[tfull[:]])
            # layer 2
            layer(tfull[:], town[:], wfs[1], rts[1], bbs[1],
                  outt.ap(), last=True)

    nc.finalize()
    return nc


# ------------------------------------------------------------------- driver

_cache = {}


def _get_program(cfg):
    key = (cfg.NW, cfg.NCHA, cfg.GW, cfg.DT, cfg.NCORES,
           cfg.UPOOL, cfg.INCPOOL)
    if key not in _cache:
        _cache[key] = build_program(cfg, cfg.NCORES)
    return _cache[key]


def run(cfg, images, edges, pseudo, W1, root1, b1, W2, root2, b2,
        trace=False, trace_out=None, tmpdir=None):
    wf = []
    for W in (W1, W2):
        Wflat = np.asarray(W, np.float32).reshape(cfg.KK * cfg.C, cfg.O)
        wfl = np.zeros((96, 96), np.float32)
        for j in range(3):
            wfl[:, 32 * j:32 * j + 32] = Wflat[96 * j:96 * j + 96, :]
        wf.append(wfl)
    rts = [np.asarray(r, np.float32).astype(_np_dt(cfg.DT))
           for r in (root1, root2)]
    bbs = [np.asarray(b, np.float32).reshape(cfg.O, 1) for b in (b1, b2)]

    in_maps = []
    sigmas = []
    for b in range(cfg.B):
        tab0, halves, sigma = _host_prep_mesh(
            cfg, np.asarray(images[b], np.float32),
            np.asarray(edges[b]), np.asarray(pseudo[b], np.float32))
        sigmas.append(sigma)
        for h in range(2):
            IDXA, IDXB, ED = halves[h]
            in_maps.append({
                "tab0": tab0,
                "xown0": tab0[h * cfg.NHALF:(h + 1) * cfg.NHALF],
                "idxa": IDXA, "idxb": IDXB,
                "edd": ED,
                "wf1": wf[0], "wf2": wf[1],
                "rt1": rts[0], "rt2": rts[1],
                "bb1": bbs[0], "bb2": bbs[1],
            })

    nc = _get_program(cfg)
    res = bass_utils.run_bass_kernel_spmd(
        nc, in_maps, core_ids=list(range(cfg.NCORES)), trace=trace,
        tmpdir=tmpdir)
    if trace_out is not None:
        trace_out.append(res)
    outs = res.results

    out = np.empty((cfg.B, cfg.N, cfg.O), np.float32)
    for b in range(cfg.B):
        full = np.concatenate([outs[2 * b]["out"], outs[2 * b + 1]["out"]],
                              axis=0)
        out[b] = full[sigmas[b]]
    return out


def kernel(images, edges, pseudo, W1, root1, b1, W2, root2, b2):
    cfg = CFG()
    return run(cfg, images, edges, pseudo, W1, root1, b1,
               W2, root2, b2)

